# revision 9
# baseline (speedup 1.0000x reference)
"""Trainium2 kernel for nn_DCT_base_Rec_Module (topk_masking).

Math (validated against the reference in numpy):
  - The level filter is all-ones (i+j <= 62 < 64) and the DCT matrix D is
    orthonormal, so level_y == patches up to fp32 roundoff (~2e-7 rel): the
    four outputs are raw 32x32 input-image windows at grade-selected indices.
  - The hardware computes the grade matrix
        grade[b,l] = sum_{c,i,j} F[i,j] * ln(|(D P_{b,l,c} D^T)[i,j]| + 1)
    with F = sum_g (w_g/ft_g) * filt_g; index selection and window slicing
    happen on the host during unsharding.

Per-core pipeline (2 images x 3 channels, pure data parallel over B):
  stage A  (PE):  WT[c,(oh,i)] = sum_R X[R,c]*D[i,R-16*oh] via banded-matrix
                  matmuls (psum-accumulated over 128-row K-tiles).
  copy     (DVE): WT psum -> SBUF.
  stage B  (PE):  per 128-wide c-chunk (offset 112m) and window-parity pass,
                  Z[(w',j),(oh,i)] = block-diag DCT taps applied to WT chunk.
  abs      (DVE copy + GPSIMD max(-z,z) for one half; ACT Abs for the other).
  ln       (ACT): lx = Ln(|Z| + 1) -> grouped SBUF tile.
  F-reduce (PE):  32 psum-accumulated matmuls over i-slices with
                  lhsT[i] = blockdiag(F[i,:]), reducing (j, i) per patch.
  ch-sum   (DVE): sum 3 channels -> grades -> DRAM.
"""
import numpy as np
from contextlib import ExitStack

import concourse.bass as bass
import concourse.tile as tile
from concourse import mybir, bacc
from concourse.bass_utils import run_bass_kernel_spmd

FP32 = mybir.dt.float32
AF = mybir.ActivationFunctionType
ALU = mybir.AluOpType

N_CORES = 8
B, C, H, W = 16, 3, 512, 512
WS, STRIDE, NH = 32, 16, 31
L = NH * NH
IMGS = B // N_CORES          # images per core (2)
NCH = IMGS * C               # image-channels per core (6)

# per-(img,ch) stage-B passes: (m, parity) with parity=None meaning the merged
# single pass for the m=4 chunk (windows ow=28,29,30 at tap offsets 0,16,32).
PASSES = [(0, 0), (0, 1), (1, 0), (1, 1), (2, 0),
          (2, 1), (3, 0), (3, 1), (4, None)]
GROUPS = [PASSES[0:5], PASSES[5:9]]          # F-reduce groups (5 and 4 slots)


# ---------------------------------------------------------------- constants
def _dct_mat(size):
    i = np.arange(size)[:, None]
    j = np.arange(size)[None, :]
    scale = np.where(i == 0, np.sqrt(1.0 / size), np.sqrt(2.0 / size))
    return (scale * np.cos((j + 0.5) * np.pi * i / size)).astype(np.float32)


def _gen_filter(start, end, size):
    i = np.arange(size)[:, None]
    j = np.arange(size)[None, :]
    s = i + j
    return np.where((s > end) | (s < start), 0.0, 1.0).astype(np.float32)


def _build_consts():
    D = _dct_mat(WS)
    G = 6
    gf = np.stack([_gen_filter(WS * 2.0 / G * g, WS * 2.0 / G * (g + 1), WS)
                   for g in range(G)])
    ftn = gf.sum(axis=(1, 2))
    wg = (2.0 ** np.arange(G)).astype(np.float32)
    F = (gf * (wg / ftn)[:, None, None]).sum(axis=0).astype(np.float32)

    # banded stage-A matrix A[R,(oh,i)] = D[i, R-16*oh] in 128-row K-tiles.
    # k=0 is stored full width (its start=True matmuls initialize the whole
    # psum tile); k>=1 store only their band columns.
    A = np.zeros((512, NH, 32), np.float32)
    for oh in range(NH):
        A[16 * oh:16 * oh + 32, oh, :] = D.T
    A = A.reshape(512, NH * 32)
    bandA0 = A[0:128].copy()                                   # [128, 992]
    bandA1 = A[128:256, 224:512].copy()                        # [128, 288] ohs 7..15
    bandA2 = A[256:384, 480:768].copy()                        # [128, 288] ohs 15..23
    bandA3 = A[384:512, 736:992].copy()                        # [128, 256] ohs 23..30

    # stage-B block-diag taps: for parity passes taps[par][cl, 32*w'+j] =
    # D[j, cl-off], off = 32*w'+16*par; the merged m=4 pass uses offsets
    # 16*w' (w'=0..2).
    taps = np.zeros((3, 128, 128), np.float32)
    for par in range(2):
        for wp in range(4):
            off = 32 * wp + 16 * par
            if off + 32 <= 128:
                taps[par, off:off + 32, wp * 32:(wp + 1) * 32] = D.T
    for wp in range(3):
        off = 16 * wp
        taps[2, off:off + 32, wp * 32:(wp + 1) * 32] = D.T

    # F-reduce weights: fmat[32*w'+j, i, w''] = F[i,j] * delta_{w',w''}
    fmat = np.zeros((128, 32, 4), np.float32)
    for wp in range(4):
        fmat[wp * 32:(wp + 1) * 32, :, wp] = F.T
    return D, F, bandA0, bandA1, bandA2, bandA3, taps, fmat


# ---------------------------------------------------------------- program
def _build_program():
    nc = bacc.Bacc("TRN2", target_bir_lowering=False, debug=False,
                   enable_asserts=True)
    xs_d = nc.dram_tensor("xs", [NCH, H, W], FP32, kind="ExternalInput").ap()
    ba0_d = nc.dram_tensor("bandA0", [128, 992], FP32, kind="ExternalInput").ap()
    ba1_d = nc.dram_tensor("bandA1", [128, 288], FP32, kind="ExternalInput").ap()
    ba2_d = nc.dram_tensor("bandA2", [128, 288], FP32, kind="ExternalInput").ap()
    ba3_d = nc.dram_tensor("bandA3", [128, 256], FP32, kind="ExternalInput").ap()
    taps_d = nc.dram_tensor("taps", [3, 128, 128], FP32, kind="ExternalInput").ap()
    fmat_d = nc.dram_tensor("fmat", [128, 32, 4], FP32, kind="ExternalInput").ap()
    # grades[img, g, w', s, oh]: group g's slot s = GROUPS[g][s] = (m, par)
    gr_d = nc.dram_tensor("grades", [IMGS, 2, 4, 5, NH], FP32,
                          kind="ExternalOutput").ap()

    with tile.TileContext(nc) as tc, ExitStack() as ctx:
        cpool = ctx.enter_context(tc.tile_pool(name="consts", bufs=1))
        xpool = ctx.enter_context(tc.tile_pool(name="x", bufs=1))
        wtpool = ctx.enter_context(tc.tile_pool(name="wtsb", bufs=2))
        azpool = ctx.enter_context(tc.tile_pool(name="az", bufs=3))
        lxpool = ctx.enter_context(tc.tile_pool(name="lx", bufs=1))
        gpool = ctx.enter_context(tc.tile_pool(name="gsum", bufs=2))
        wtps = ctx.enter_context(tc.tile_pool(name="wtps", bufs=1, space="PSUM"))
        zps = ctx.enter_context(tc.tile_pool(name="zps", bufs=2, space="PSUM"))
        frps = ctx.enter_context(tc.tile_pool(name="frps", bufs=1, space="PSUM"))

        # constants -> SBUF
        ba0 = cpool.tile([128, 992], FP32, tag="ba0", name="ba0")
        nc.sync.dma_start(ba0[:], ba0_d[:])
        ba1 = cpool.tile([128, 288], FP32, tag="ba1", name="ba1")
        nc.sync.dma_start(ba1[:], ba1_d[:])
        ba2 = cpool.tile([128, 288], FP32, tag="ba2", name="ba2")
        nc.sync.dma_start(ba2[:], ba2_d[:])
        ba3 = cpool.tile([128, 256], FP32, tag="ba3", name="ba3")
        nc.sync.dma_start(ba3[:], ba3_d[:])
        taps = cpool.tile([128, 3, 128], FP32, tag="taps", name="taps")
        for tp in range(3):
            nc.sync.dma_start(taps[:, tp, :], taps_d[tp])
        fmat = cpool.tile([128, 32, 4], FP32, tag="fmat", name="fmat")
        nc.sync.dma_start(fmat[:], fmat_d[:])

        for img in range(IMGS):
            xt = []
            for ch in range(C):
                t = xpool.tile([128, 4, 512], FP32, tag=f"x{ch}",
                               name=f"x_{img}_{ch}")
                for k in range(4):
                    nc.sync.dma_start(t[:, k, :],
                                      xs_d[img * C + ch, 128 * k:128 * k + 128, :])
                xt.append(t)

            lx_g = [lxpool.tile([128, len(GROUPS[gi]) * 3, NH, 32], FP32,
                                tag=f"lx{gi}", name=f"lx_{img}_{gi}")
                    for gi in range(2)]

            for m in range(5):
                co = 112 * m
                cw = 128 if m < 4 else 64
                for ch in range(C):
                    x_t = xt[ch]
                    wt0 = wtps.tile([128, 512], FP32, tag="wt0", name="wt0")
                    wt1 = wtps.tile([128, 480], FP32, tag="wt1", name="wt1")
                    lhs0 = x_t[:, 0, co:co + cw]
                    nc.tensor.matmul(wt0[0:cw, :], lhs0, ba0[:, 0:512],
                                     start=True, stop=False, skip_group_check=True)
                    nc.tensor.matmul(wt1[0:cw, :], lhs0, ba0[:, 512:992],
                                     start=True, stop=False, skip_group_check=True)
                    nc.tensor.matmul(wt0[0:cw, 224:512], x_t[:, 1, co:co + cw],
                                     ba1[:], start=False, stop=False,
                                     skip_group_check=True)
                    nc.tensor.matmul(wt0[0:cw, 480:512], x_t[:, 2, co:co + cw],
                                     ba2[:, 0:32], start=False, stop=True,
                                     skip_group_check=True)
                    nc.tensor.matmul(wt1[0:cw, 0:256], x_t[:, 2, co:co + cw],
                                     ba2[:, 32:288], start=False, stop=False,
                                     skip_group_check=True)
                    nc.tensor.matmul(wt1[0:cw, 224:480], x_t[:, 3, co:co + cw],
                                     ba3[:], start=False, stop=True,
                                     skip_group_check=True)

                    wts = wtpool.tile([128, 992], FP32, tag="wts", name="wts")
                    nc.vector.tensor_copy(wts[0:cw, 0:512], wt0[0:cw, :])
                    nc.vector.tensor_copy(wts[0:cw, 512:992], wt1[0:cw, :])
                    if m == 4:
                        nc.gpsimd.memset(wts[64:128, :], 0.0)

                    for par in ([0, 1] if m < 4 else [None]):
                        pi = PASSES.index((m, par))
                        g, s = (0, pi) if pi < 5 else (1, pi - 5)
                        sc = s * 3 + ch
                        tp = par if par is not None else 2
                        z0 = zps.tile([128, 512], FP32, tag="z0", name="z0")
                        z1 = zps.tile([128, 480], FP32, tag="z1", name="z1")
                        nc.tensor.matmul(z0[:], taps[:, tp, :], wts[:, 0:512],
                                         start=True, stop=True)
                        nc.tensor.matmul(z1[:], taps[:, tp, :], wts[:, 512:992],
                                         start=True, stop=True)
                        az = azpool.tile([128, 992], FP32, tag="az", name="az")
                        nc.scalar.activation(az[:, 0:512], z0[:], AF.Abs)
                        nc.scalar.activation(az[:, 512:992], z1[:], AF.Abs)
                        out_ap = lx_g[g][:, sc, :, :].rearrange("p a b -> p (a b)")
                        nc.scalar.activation(out_ap, az[:], AF.Ln, bias=1.0)

            # F-reduce per group: 32 psum-accumulated i-slice matmuls
            for g in range(2):
                ns = len(GROUPS[g]) * 3
                fr = frps.tile([4, ns // 3, 3, NH], FP32, tag=f"fr{g}",
                               name=f"fr_{img}_{g}")
                for i in range(32):
                    nc.tensor.matmul(fr[:], fmat[:, i, :], lx_g[g][:, :, :, i],
                                     start=(i == 0), stop=(i == 31))
                tmp = gpool.tile([4, ns // 3, NH], FP32, tag=f"gtmp{g}",
                                 name=f"gtmp_{img}_{g}")
                nc.vector.tensor_copy(tmp[:], fr[:, :, 0, :])
                tmp2 = gpool.tile([4, ns // 3, NH], FP32, tag=f"gtmp2{g}",
                                  name=f"gtmp2_{img}_{g}")
                nc.vector.tensor_add(tmp2[:], tmp[:], fr[:, :, 1, :])
                gsum = gpool.tile([4, ns // 3, NH], FP32, tag=f"gsum{g}",
                                  name=f"gsum_{img}_{g}")
                nc.vector.tensor_add(gsum[:], tmp2[:], fr[:, :, 2, :])
                nc.sync.dma_start(gr_d[img, g, :, 0:ns // 3, :], gsum[:])

    nc.compile()
    return nc


_PROGRAM_CACHE = {}


def _get_program():
    if "nc" not in _PROGRAM_CACHE:
        _PROGRAM_CACHE["nc"] = _build_program()
    return _PROGRAM_CACHE["nc"]


def _make_in_maps(x):
    _, _, ba0, ba1, ba2, ba3, taps, fmat = _build_consts()
    in_maps = []
    for c in range(N_CORES):
        in_maps.append({
            "xs": np.ascontiguousarray(
                x[c * IMGS:(c + 1) * IMGS].reshape(NCH, H, W)),
            "bandA0": ba0, "bandA1": ba1, "bandA2": ba2, "bandA3": ba3,
            "taps": taps, "fmat": fmat,
        })
    return in_maps


def _grades_from_results(results):
    grade = np.full((B, L), np.nan, np.float32)
    for c in range(N_CORES):
        gr = results[c]["grades"]              # [IMGS, 2, 4, 5, NH]
        for img in range(IMGS):
            b = c * IMGS + img
            for g in range(2):
                for s, (m, par) in enumerate(GROUPS[g]):
                    for wp in range(4):
                        if par is None:        # merged m=4 pass
                            if wp > 2:
                                continue
                            ow = 7 * m + wp
                        else:
                            wl = 2 * wp + par
                            if wl > 6:
                                continue
                            ow = 7 * m + wl
                        if ow >= NH:
                            continue
                        grade[b, np.arange(NH) * NH + ow] = gr[img, g, wp, s, :]
    assert not np.isnan(grade).any()
    return grade


# ---------------------------------------------------------------- entry point
def kernel(x: np.ndarray) -> tuple:
    x = np.ascontiguousarray(np.asarray(x, dtype=np.float32))
    assert x.shape == (B, C, H, W)

    nc = _get_program()
    res = run_bass_kernel_spmd(nc, _make_in_maps(x), core_ids=list(range(N_CORES)))
    grade = _grades_from_results(res.results)
    idx = np.argsort(grade, axis=-1)

    def pick(sel):
        out = np.empty((B, C, WS, WS), np.float32)
        for b in range(B):
            oh, ow = divmod(int(sel[b]), NH)
            out[b] = x[b, :, STRIDE * oh:STRIDE * oh + WS,
                       STRIDE * ow:STRIDE * ow + WS]
        return out

    return (pick(idx[:, 0]), pick(idx[:, -1]), pick(idx[:, 1]), pick(idx[:, -2]))
